# revision 48
# baseline (speedup 1.0000x reference)
"""KNIFE entropy regularizer loss on 8 Trainium2 NeuronCores.

reference math (per token n, center k):
    dist_sq[n,k] = max(||x_n||^2 + ||c_k||^2 - 2 x_n.c_k, 0)
    kv[n,k]      = exp(-dist_sq / (2 s_k^2))
    density[n]   = sum_k w_k kv[n,k]
    h            = -mean_n log(density + EPS)
    out          = [BETA*h, (h-TGT)^2, BETA*h + (h-TGT)^2, h]

Sharding: data-parallel over the flattened token axis N = B*S = 8192,
1024 tokens per core, shard pre-transposed to [H=1024, T=1024] so the
contraction axis lands on SBUF partitions.  Tiny params are host-folded:
nu_k = -1/(2 s_k^2), bias_k = nu_k*||c_k||^2 + ln w_k, and -2c packed in
the [128, j, k] chunk layout the PE weights want.

Device pipeline per core (~30us vs the 33us cast-DMA baseline; wire
measured at 376GB/s aggregate over the 16 DMA engines):
  - 11 raw fp32 HWDGE DMAs on the SP queue (7 full 128-row H-chunks +
    the last chunk in 4 token-pieces [512/256/128/128] so the tail
    pipelines): no cast in flight, no SWDGE descriptor-gen serialization.
  - PE: psum[k,t] accumulates -2c.x via fp32r matmuls (full rate at
    >=256 moving columns, no input casts) and ||x||^2 via bf16
    ones-matmuls over DVE-squared chunks; PSUM is split into one tile
    per token-piece (whole-tile dep tracking would otherwise serialize
    the epilogue behind the last matmul); 5 filler matmuls after chunk 0
    hold the HAM clock ramp through the early DMA gap.
  - ACT/DVE: raw dist rows copied PSUM -> SBUF bf16 with a ones-row;
    the [K+1, K] transpose-matmul weights [diag(nu); nu*csq + ln w]
    apply the exp affine during the PE transpose to [128t, blk, k].
  - ACT: exp on [128, 80] (80 columns instead of 2x512), one table load
    (pre-placed InstLoadActFuncSet of the combined exp+ln table set).
  - DVE: density[t, blk] = free-axis reduce over k.
  - ACT: ln(density + EPS) on [128, 8] with per-partition accum_out.
  - PE: ones-matmul partition-sum -> [1,1]; the out DMA is ONE
    descriptor (a [128,1] out spends ~7us on per-descriptor completion
    semaphores at kernel end).
The max(dist,0) clamp is a no-op for this regime (dist ~ 1e3, exp
underflows to 0 either way) and is elided.
"""

from contextlib import ExitStack

import numpy as np

import concourse.bass as bass
import concourse.tile as tile
from concourse import bacc, mybir
from concourse.bass_utils import run_bass_kernel_spmd
from concourse.hw_specs import get_activation_tables

B, S, H, K = 4, 2048, 1024, 10
N = B * S                      # 8192 tokens
NCORES = 8
TPC = N // NCORES              # 1024 tokens per core
HCHUNKS = H // 128             # 8 chunks of 128 partitions
HALF = 512
NBLK = TPC // 128              # 8 token blocks for the transposed epilogue
BETA = 1.0
TARGET_ENTROPY = 0.0
EPS = 1e-8

F32 = mybir.dt.float32
F32R = mybir.dt.float32r
BF16 = mybir.dt.bfloat16



def _build_program():
    nc = bacc.Bacc("TRN2", target_bir_lowering=False, debug=False,
                   num_devices=NCORES)

    # x and -2c live as float32r end-to-end (same bits as fp32) so the PE
    # can consume them at full rate without a cast pass
    xT = nc.dram_tensor("xT", [H, TPC], F32R, kind="ExternalInput").ap()
    pc2 = nc.dram_tensor("pc2", [128, HCHUNKS * K], F32R,
                         kind="ExternalInput").ap()
    pf = nc.dram_tensor("pf", [K + 1, K], BF16, kind="ExternalInput").ap()
    out = nc.dram_tensor("out", [1, 1], F32, kind="ExternalOutput").ap()

    with tile.TileContext(nc) as tc, ExitStack() as ctx:
        _emit(tc, ctx, xT, pc2, pf, out)
    nc.compile()
    return nc


def _emit(tc, ctx, xT, pc2, pf, out):
    nc = tc.nc
    singles = ctx.enter_context(tc.tile_pool(name="singles", bufs=1))
    xpool = ctx.enter_context(tc.tile_pool(name="x", bufs=1))
    sqpool = ctx.enter_context(tc.tile_pool(name="sq", bufs=1))
    psum = ctx.enter_context(tc.tile_pool(name="ps", bufs=1, space="PSUM"))

    # ---- ACT: pin the combined exp+ln table once, before any activation;
    # Bacc.insert_act_table_loads honors the pre-placed load so neither Exp
    # nor Ln charges an implicit 1.3us table switch on the critical path ----
    table_id = list(get_activation_tables(nc.m.arch)).index(
        "natural_log_exp_and_others")
    nc.scalar.add_instruction(mybir.InstLoadActFuncSet(
        name=nc.get_next_instruction_name(), ins=[], outs=[],
        act_func_set_id=table_id))

    # ---- params: two SWDGE DMAs off the hot SP queue ----
    c2_sb = singles.tile([128, HCHUNKS, K], F32R)
    nc.gpsimd.dma_start(c2_sb[:], pc2.rearrange("p (j k) -> p j k", k=K))
    ptw_sb = singles.tile([K + 1, K], BF16)
    nc.gpsimd.dma_start(ptw_sb[:], pf[:, :])
    c2 = c2_sb

    # ---- constants ----
    warm_rhs = singles.tile([128, HALF], BF16)
    nc.vector.memset(warm_rhs[:], 0.0)
    ones_bf = singles.tile([128, K], BF16)
    nc.vector.memset(ones_bf[:], 1.0)
    eps_sb = singles.tile([128, 1], F32)
    nc.vector.memset(eps_sb[:], EPS)
    zero_sb = singles.tile([128, 1], F32)
    nc.vector.memset(zero_sb[:], 0.0)

    # ---- x stream: 10 raw fp32 DMAs on the SP HWDGE queue.  Chunks 0-6
    # full [128, 1024]; chunk 7 in three token pieces so the tail of the
    # wire pipelines into the epilogue ----
    xb = [xpool.tile([128, TPC], F32R, name=f"xb_{j}", tag=f"x{j}")
          for j in range(HCHUNKS)]
    last = HCHUNKS - 1
    pieces = [(0, 512), (512, 768), (768, 896), (896, TPC)]
    for j in range(last):
        nc.sync.dma_start(xb[j][:], xT[j * 128:(j + 1) * 128, :])
    for (t0, t1) in pieces:
        nc.sync.dma_start(xb[last][:, t0:t1], xT[last * 128:, t0:t1])



    # ---- PE warmup: bridge idle -> data-gated stream at full clock ----
    ps_warm = psum.tile([K, HALF], F32)
    for _ in range(7):
        nc.tensor.matmul(ps_warm[:], lhsT=ones_bf[:], rhs=warm_rhs[:],
                         start=True, stop=True)

    # ---- main accumulation: psum[k, t] = x_sq[t] - 2 dot[k, t].
    # One PSUM tile per token region so each region's consumers release as
    # soon as its own writers finish (whole-tile dep tracking otherwise
    # serializes the epilogue behind the last matmul) ----
    psd = [psum.tile([16, t1 - t0], F32, name=f"psd_{i}")
           for i, (t0, t1) in enumerate(pieces)]

    def mm(out_ap, lhsT, rhs, **kw):
        nc.tensor.matmul(out_ap, lhsT=lhsT, rhs=rhs, skip_group_check=True,
                         **kw)

    sq = [sqpool.tile([128, TPC], BF16, name=f"sq_{j}", tag=f"s{j}")
          for j in range(HCHUNKS)]
    for j in range(last):
        xf = xb[j][:].bitcast(F32)
        nc.vector.tensor_mul(sq[j][:], xf, xf)
        c2j = c2[:, j, :]
        for i, (t0, t1) in enumerate(pieces):
            mm(psd[i][0:K, :], c2j, xb[j][:, t0:t1],
               start=(j == 0), stop=False)
        for i, (t0, t1) in enumerate(pieces):
            mm(psd[i][0:K, :], ones_bf[:], sq[j][:, t0:t1],
               start=False, stop=False)
        if j == 0:
            # the wire delivers chunk 1 ~2.4us after chunk 0; keep the PE
            # busy through the gap so the HAM clock ramp reaches 2.4GHz
            # instead of resetting (idle PE falls back to 1.2GHz and the
            # whole stream + tail then runs at half rate)
            for _ in range(5):
                nc.tensor.matmul(ps_warm[:], lhsT=ones_bf[:],
                                 rhs=warm_rhs[:], start=True, stop=True)

    # last chunk: per-piece so compute overlaps the DMA tail
    c2l = c2[:, last, :]
    for i, (t0, t1) in enumerate(pieces):
        sl = slice(t0, t1)
        xf = xb[last][:, sl].bitcast(F32)
        nc.vector.tensor_mul(sq[last][:, sl], xf, xf)
        mm(psd[i][0:K, :], c2l, xb[last][:, sl],
           start=False, stop=False)
        mm(psd[i][0:K, :], ones_bf[:], sq[last][:, sl],
           start=False, stop=True)

    # ---- epilogue, token-major.  The raw dist rows are copied PSUM->SBUF
    # in bf16 with a ones-row appended; the transpose matmul's weights
    # [K+1, K] = [diag(ninv); bias] then compute z = ninv*dist + bias
    # (bias = ninv*csq + ln w) as part of the transpose itself ----
    zd = [singles.tile([K + 1, t1 - t0], BF16, name=f"zd_{i}")
          for i, (t0, t1) in enumerate(pieces)]
    for i in range(len(pieces)):
        nc.vector.memset(zd[i][:], 1.0)      # row K stays the ones-row
    # pieces 0-1 copy on ACT (idle until the exps), 2-3 on DVE: neither
    # engine serializes more than ~0.6us of PSUM->SBUF copy on the tail
    nc.scalar.copy(zd[0][0:K, :], psd[0][0:K, :])
    nc.scalar.copy(zd[1][0:K, :], psd[1][0:K, :])
    nc.vector.tensor_copy(zd[2][0:K, :], psd[2][0:K, :])
    nc.vector.tensor_copy(zd[3][0:K, :], psd[3][0:K, :])

    # transpose z to [128t, blk, k] via tiny identity matmuls; exp and the
    # density reduce run per piece-group so piece 0-1 epilogue work is not
    # ordered behind piece 2's transposes (whole-tile dep tracking)
    psT = psum.tile([128, NBLK, K], F32)
    kvT = singles.tile([128, NBLK, K], F32)
    dens = singles.tile([128, NBLK], F32)
    blocks = [(t0 // 128, t1 // 128) for (t0, t1) in pieces]
    for i, (b0, b1) in enumerate(blocks):
        for b in range(b0, b1):
            o = (b - b0) * 128
            mm(psT[:, b, :], zd[i][:, o:o + 128], ptw_sb[:],
               start=True, stop=True)
        if b1 == 6:
            nc.scalar.activation(kvT[:, 0:6, :], psT[:, 0:6, :],
                                 mybir.ActivationFunctionType.Exp,
                                 bias=zero_sb[:])
            nc.vector.tensor_reduce(dens[:, 0:6], kvT[:, 0:6, :],
                                    axis=mybir.AxisListType.X,
                                    op=mybir.AluOpType.add)
    nc.scalar.activation(kvT[:, 6:NBLK, :], psT[:, 6:NBLK, :],
                         mybir.ActivationFunctionType.Exp, bias=zero_sb[:])
    nc.vector.tensor_reduce(dens[:, 6:NBLK], kvT[:, 6:NBLK, :],
                            axis=mybir.AxisListType.X, op=mybir.AluOpType.add)

    # ln(density + eps), accumulated along the free axis per partition
    lnout = singles.tile([128, NBLK], F32)
    lnacc = singles.tile([128, 1], F32)
    nc.scalar.activation(lnout[:], dens[:], mybir.ActivationFunctionType.Ln,
                         bias=eps_sb[:], accum_out=lnacc[:])

    # partition-sum on the PE so the output DMA is ONE descriptor (a
    # [128, 1] DMA is 128 4-byte descriptors whose completion-semaphore
    # processing alone costs ~7us at kernel end); fp32 keeps the partial
    # sums exact (bf16 here costs a 5e-3 systematic rounding bias)
    ones_f32 = singles.tile([128, 1], F32)
    nc.vector.memset(ones_f32[:], 1.0)
    ps_sum = psum.tile([1, 1], F32)
    nc.tensor.matmul(ps_sum[:], lhsT=lnacc[:], rhs=ones_f32[:],
                     start=True, stop=True)
    total_sb = singles.tile([1, 1], F32)
    nc.vector.tensor_copy(total_sb[:], ps_sum[:])
    nc.scalar.dma_start(out[:, :], total_sb[:])


def _make_in_maps(hidden_states, kernel_centers, kernel_weights, kernel_scales):
    h_flat = np.asarray(hidden_states, dtype=np.float32).reshape(N, H)
    c = np.asarray(kernel_centers, np.float64)
    w = np.asarray(kernel_weights, np.float64).reshape(K)
    s = np.asarray(kernel_scales, np.float64).reshape(K)

    ninv = -1.0 / (2.0 * s * s)
    csq = np.sum(c * c, axis=1)                       # [K]
    with np.errstate(divide="ignore"):
        lnw = np.log(w)                               # w=0 -> -inf -> e^z=0
    zbias = ninv * csq + lnw

    # [p, j*K+k] = -2 c[k, j*128+p]
    pc2 = np.ascontiguousarray(
        (-2.0 * c.T).reshape(HCHUNKS, 128, K).transpose(1, 0, 2).reshape(
            128, HCHUNKS * K).astype(np.float32))
    # transpose-matmul weights: out[t, k] = dist[k, t]*ninv_k + bias_k
    import ml_dtypes
    pf = np.zeros((K + 1, K), np.float32)
    pf[:K, :] = np.diag(ninv)
    pf[K, :] = zbias
    pf = pf.astype(ml_dtypes.bfloat16)

    in_maps = []
    for core in range(NCORES):
        shard = h_flat[core * TPC:(core + 1) * TPC, :]    # [TPC, H]
        in_maps.append({
            "xT": np.ascontiguousarray(shard.T),          # [H, TPC]
            "pc2": pc2,
            "pf": pf,
        })
    return in_maps


def run(inputs, trace=False, **run_kwargs):
    """Compile + run on 8 cores. Returns (output[4], BassKernelResults)."""
    nc = _build_program()
    in_maps = _make_in_maps(**inputs)
    results = run_bass_kernel_spmd(
        nc, in_maps, core_ids=list(range(NCORES)), trace=trace, **run_kwargs)
    partial = np.float64(0.0)
    for r in results.results:
        partial += np.float64(r["out"][0, 0])
    h = np.float32(-(partial / N))
    entropy_loss = np.float32(BETA) * h
    target_entropy_loss = np.float32((h - TARGET_ENTROPY) ** 2)
    total_loss = entropy_loss + target_entropy_loss
    outv = np.stack([entropy_loss, target_entropy_loss, total_loss, h]).astype(
        np.float32)
    return outv, results


def kernel(**inputs):
    outv, _ = run(inputs, trace=False)
    return outv


# revision 49
# speedup vs baseline: 1.1349x; 1.1349x over previous
"""KNIFE entropy regularizer loss on 8 Trainium2 NeuronCores.

reference math (per token n, center k):
    dist_sq[n,k] = max(||x_n||^2 + ||c_k||^2 - 2 x_n.c_k, 0)
    kv[n,k]      = exp(-dist_sq / (2 s_k^2))
    density[n]   = sum_k w_k kv[n,k]
    h            = -mean_n log(density + EPS)
    out          = [BETA*h, (h-TGT)^2, BETA*h + (h-TGT)^2, h]

Sharding: data-parallel over the flattened token axis N = B*S = 8192,
1024 tokens per core, shard pre-transposed to [H=1024, T=1024] so the
contraction axis lands on SBUF partitions.  Tiny params are host-folded:
nu_k = -1/(2 s_k^2), bias_k = nu_k*||c_k||^2 + ln w_k, and -2c packed in
the [128, j, k] chunk layout the PE weights want.

Device pipeline per core (~30us vs the 33us cast-DMA baseline; wire
measured at 376GB/s aggregate over the 16 DMA engines):
  - 11 raw fp32 HWDGE DMAs on the SP queue (7 full 128-row H-chunks +
    the last chunk in 4 token-pieces [512/256/128/128] so the tail
    pipelines): no cast in flight, no SWDGE descriptor-gen serialization.
  - PE: psum[k,t] accumulates -2c.x via fp32r matmuls (full rate at
    >=256 moving columns, no input casts) and ||x||^2 via bf16
    ones-matmuls over DVE-squared chunks; PSUM is split into one tile
    per token-piece (whole-tile dep tracking would otherwise serialize
    the epilogue behind the last matmul); 5 filler matmuls after chunk 0
    hold the HAM clock ramp through the early DMA gap.
  - ACT/DVE: raw dist rows copied PSUM -> SBUF bf16 with a ones-row;
    the [K+1, K] transpose-matmul weights [diag(nu); nu*csq + ln w]
    apply the exp affine during the PE transpose to [128t, blk, k].
  - ACT: exp on [128, 80] (80 columns instead of 2x512), one table load
    (pre-placed InstLoadActFuncSet of the combined exp+ln table set).
  - DVE: density[t, blk] = free-axis reduce over k.
  - ACT: ln(density + EPS) on [128, 8] with per-partition accum_out.
  - PE: ones-matmul partition-sum -> [1,1]; the out DMA is ONE
    descriptor (a [128,1] out spends ~7us on per-descriptor completion
    semaphores at kernel end).
The max(dist,0) clamp is a no-op for this regime (dist ~ 1e3, exp
underflows to 0 either way) and is elided.
"""

from contextlib import ExitStack

import numpy as np

import concourse.bass as bass
import concourse.tile as tile
from concourse import bacc, mybir
from concourse.bass_utils import run_bass_kernel_spmd
from concourse.hw_specs import get_activation_tables

B, S, H, K = 4, 2048, 1024, 10
N = B * S                      # 8192 tokens
NCORES = 8
TPC = N // NCORES              # 1024 tokens per core
HCHUNKS = H // 128             # 8 chunks of 128 partitions
HALF = 512
NBLK = TPC // 128              # 8 token blocks for the transposed epilogue
BETA = 1.0
TARGET_ENTROPY = 0.0
EPS = 1e-8

F32 = mybir.dt.float32
F32R = mybir.dt.float32r
BF16 = mybir.dt.bfloat16



def _build_program():
    nc = bacc.Bacc("TRN2", target_bir_lowering=False, debug=False,
                   num_devices=NCORES)

    # x and -2c live as float32r end-to-end (same bits as fp32) so the PE
    # can consume them at full rate without a cast pass
    xT = nc.dram_tensor("xT", [H, TPC], F32R, kind="ExternalInput").ap()
    pc2 = nc.dram_tensor("pc2", [128, HCHUNKS * K], F32R,
                         kind="ExternalInput").ap()
    pf = nc.dram_tensor("pf", [K + 1, K], BF16, kind="ExternalInput").ap()
    out = nc.dram_tensor("out", [1, 1], F32, kind="ExternalOutput").ap()

    with tile.TileContext(nc) as tc, ExitStack() as ctx:
        _emit(tc, ctx, xT, pc2, pf, out)
    nc.compile()
    return nc


def _emit(tc, ctx, xT, pc2, pf, out):
    nc = tc.nc
    singles = ctx.enter_context(tc.tile_pool(name="singles", bufs=1))
    xpool = ctx.enter_context(tc.tile_pool(name="x", bufs=1))
    sqpool = ctx.enter_context(tc.tile_pool(name="sq", bufs=1))
    psum = ctx.enter_context(tc.tile_pool(name="ps", bufs=1, space="PSUM"))

    # ---- ACT: pin the combined exp+ln table once, before any activation;
    # Bacc.insert_act_table_loads honors the pre-placed load so neither Exp
    # nor Ln charges an implicit 1.3us table switch on the critical path ----
    table_id = list(get_activation_tables(nc.m.arch)).index(
        "natural_log_exp_and_others")
    nc.scalar.add_instruction(mybir.InstLoadActFuncSet(
        name=nc.get_next_instruction_name(), ins=[], outs=[],
        act_func_set_id=table_id))

    # ---- params on the ACT HWDGE queue: it carries no other DMAs, so
    # the completion semaphores process immediately (on the gpsimd SWDGE
    # queue they starve ~2us behind the x flood, delaying the first dot
    # matmul past chunk-0's arrival and resetting the PE clock ramp) ----
    c2_sb = singles.tile([128, HCHUNKS, K], F32R)
    nc.scalar.dma_start(c2_sb[:], pc2.rearrange("p (j k) -> p j k", k=K))
    ptw_sb = singles.tile([K + 1, K], BF16)
    nc.scalar.dma_start(ptw_sb[:], pf[:, :])
    c2 = c2_sb

    # ---- constants ----
    warm_rhs = singles.tile([128, HALF], BF16)
    nc.vector.memset(warm_rhs[:], 0.0)
    ones_bf = singles.tile([128, K], BF16)
    nc.vector.memset(ones_bf[:], 1.0)
    eps_sb = singles.tile([128, 1], F32)
    nc.vector.memset(eps_sb[:], EPS)
    zero_sb = singles.tile([128, 1], F32)
    nc.vector.memset(zero_sb[:], 0.0)

    # ---- x stream: 10 raw fp32 DMAs on the SP HWDGE queue.  Chunks 0-6
    # full [128, 1024]; chunk 7 in three token pieces so the tail of the
    # wire pipelines into the epilogue ----
    xb = [xpool.tile([128, TPC], F32R, name=f"xb_{j}", tag=f"x{j}")
          for j in range(HCHUNKS)]
    last = HCHUNKS - 1
    pieces = [(0, 512), (512, 768), (768, 896), (896, TPC)]
    for j in range(last):
        nc.sync.dma_start(xb[j][:], xT[j * 128:(j + 1) * 128, :])
    for (t0, t1) in pieces:
        nc.sync.dma_start(xb[last][:, t0:t1], xT[last * 128:, t0:t1])



    # ---- PE warmup: bridge idle -> data-gated stream at full clock ----
    ps_warm = psum.tile([K, HALF], F32)
    for _ in range(7):
        nc.tensor.matmul(ps_warm[:], lhsT=ones_bf[:], rhs=warm_rhs[:],
                         start=True, stop=True)

    # ---- main accumulation: psum[k, t] = x_sq[t] - 2 dot[k, t].
    # One PSUM tile per token region so each region's consumers release as
    # soon as its own writers finish (whole-tile dep tracking otherwise
    # serializes the epilogue behind the last matmul) ----
    psd = [psum.tile([16, t1 - t0], F32, name=f"psd_{i}")
           for i, (t0, t1) in enumerate(pieces)]

    def mm(out_ap, lhsT, rhs, **kw):
        nc.tensor.matmul(out_ap, lhsT=lhsT, rhs=rhs, skip_group_check=True,
                         **kw)

    sq = [sqpool.tile([128, TPC], BF16, name=f"sq_{j}", tag=f"s{j}")
          for j in range(HCHUNKS)]
    for j in range(last):
        xf = xb[j][:].bitcast(F32)
        nc.vector.tensor_mul(sq[j][:], xf, xf)
        c2j = c2[:, j, :]
        for i, (t0, t1) in enumerate(pieces):
            mm(psd[i][0:K, :], c2j, xb[j][:, t0:t1],
               start=(j == 0), stop=False)
        for i, (t0, t1) in enumerate(pieces):
            mm(psd[i][0:K, :], ones_bf[:], sq[j][:, t0:t1],
               start=False, stop=False)
        if j == 0:
            # the wire delivers chunk 1 ~2.4us after chunk 0; keep the PE
            # busy through the gap so the HAM clock ramp reaches 2.4GHz
            # instead of resetting (idle PE falls back to 1.2GHz and the
            # whole stream + tail then runs at half rate)
            for _ in range(5):
                nc.tensor.matmul(ps_warm[:], lhsT=ones_bf[:],
                                 rhs=warm_rhs[:], start=True, stop=True)

    # last chunk: per-piece so compute overlaps the DMA tail
    c2l = c2[:, last, :]
    for i, (t0, t1) in enumerate(pieces):
        sl = slice(t0, t1)
        xf = xb[last][:, sl].bitcast(F32)
        nc.vector.tensor_mul(sq[last][:, sl], xf, xf)
        mm(psd[i][0:K, :], c2l, xb[last][:, sl],
           start=False, stop=False)
        mm(psd[i][0:K, :], ones_bf[:], sq[last][:, sl],
           start=False, stop=True)

    # ---- epilogue, token-major.  The raw dist rows are copied PSUM->SBUF
    # in bf16 with a ones-row appended; the transpose matmul's weights
    # [K+1, K] = [diag(ninv); bias] then compute z = ninv*dist + bias
    # (bias = ninv*csq + ln w) as part of the transpose itself ----
    zd = [singles.tile([K + 1, t1 - t0], BF16, name=f"zd_{i}")
          for i, (t0, t1) in enumerate(pieces)]
    for i in range(len(pieces)):
        nc.vector.memset(zd[i][:], 1.0)      # row K stays the ones-row
    # pieces 0-1 copy on ACT (idle until the exps), 2-3 on DVE: neither
    # engine serializes more than ~0.6us of PSUM->SBUF copy on the tail
    nc.scalar.copy(zd[0][0:K, :], psd[0][0:K, :])
    nc.scalar.copy(zd[1][0:K, :], psd[1][0:K, :])
    nc.vector.tensor_copy(zd[2][0:K, :], psd[2][0:K, :])
    nc.vector.tensor_copy(zd[3][0:K, :], psd[3][0:K, :])

    # transpose z to [128t, blk, k] via tiny identity matmuls; exp and the
    # density reduce run per piece-group so piece 0-1 epilogue work is not
    # ordered behind piece 2's transposes (whole-tile dep tracking)
    psT = psum.tile([128, NBLK, K], F32)
    kvT = singles.tile([128, NBLK, K], F32)
    dens = singles.tile([128, NBLK], F32)
    blocks = [(t0 // 128, t1 // 128) for (t0, t1) in pieces]
    for i, (b0, b1) in enumerate(blocks):
        for b in range(b0, b1):
            o = (b - b0) * 128
            mm(psT[:, b, :], zd[i][:, o:o + 128], ptw_sb[:],
               start=True, stop=True)
        if b1 == 6:
            nc.scalar.activation(kvT[:, 0:6, :], psT[:, 0:6, :],
                                 mybir.ActivationFunctionType.Exp,
                                 bias=zero_sb[:])
            nc.vector.tensor_reduce(dens[:, 0:6], kvT[:, 0:6, :],
                                    axis=mybir.AxisListType.X,
                                    op=mybir.AluOpType.add)
    nc.scalar.activation(kvT[:, 6:NBLK, :], psT[:, 6:NBLK, :],
                         mybir.ActivationFunctionType.Exp, bias=zero_sb[:])
    nc.vector.tensor_reduce(dens[:, 6:NBLK], kvT[:, 6:NBLK, :],
                            axis=mybir.AxisListType.X, op=mybir.AluOpType.add)

    # ln(density + eps), accumulated along the free axis per partition
    lnout = singles.tile([128, NBLK], F32)
    lnacc = singles.tile([128, 1], F32)
    nc.scalar.activation(lnout[:], dens[:], mybir.ActivationFunctionType.Ln,
                         bias=eps_sb[:], accum_out=lnacc[:])

    # partition-sum on the PE so the output DMA is ONE descriptor (a
    # [128, 1] DMA is 128 4-byte descriptors whose completion-semaphore
    # processing alone costs ~7us at kernel end); fp32 keeps the partial
    # sums exact (bf16 here costs a 5e-3 systematic rounding bias)
    ones_f32 = singles.tile([128, 1], F32)
    nc.vector.memset(ones_f32[:], 1.0)
    ps_sum = psum.tile([1, 1], F32)
    nc.tensor.matmul(ps_sum[:], lhsT=lnacc[:], rhs=ones_f32[:],
                     start=True, stop=True)
    total_sb = singles.tile([1, 1], F32)
    nc.vector.tensor_copy(total_sb[:], ps_sum[:])
    nc.scalar.dma_start(out[:, :], total_sb[:])


def _make_in_maps(hidden_states, kernel_centers, kernel_weights, kernel_scales):
    h_flat = np.asarray(hidden_states, dtype=np.float32).reshape(N, H)
    c = np.asarray(kernel_centers, np.float64)
    w = np.asarray(kernel_weights, np.float64).reshape(K)
    s = np.asarray(kernel_scales, np.float64).reshape(K)

    ninv = -1.0 / (2.0 * s * s)
    csq = np.sum(c * c, axis=1)                       # [K]
    with np.errstate(divide="ignore"):
        lnw = np.log(w)                               # w=0 -> -inf -> e^z=0
    zbias = ninv * csq + lnw

    # [p, j*K+k] = -2 c[k, j*128+p]
    pc2 = np.ascontiguousarray(
        (-2.0 * c.T).reshape(HCHUNKS, 128, K).transpose(1, 0, 2).reshape(
            128, HCHUNKS * K).astype(np.float32))
    # transpose-matmul weights: out[t, k] = dist[k, t]*ninv_k + bias_k
    import ml_dtypes
    pf = np.zeros((K + 1, K), np.float32)
    pf[:K, :] = np.diag(ninv)
    pf[K, :] = zbias
    pf = pf.astype(ml_dtypes.bfloat16)

    in_maps = []
    for core in range(NCORES):
        shard = h_flat[core * TPC:(core + 1) * TPC, :]    # [TPC, H]
        in_maps.append({
            "xT": np.ascontiguousarray(shard.T),          # [H, TPC]
            "pc2": pc2,
            "pf": pf,
        })
    return in_maps


def run(inputs, trace=False, **run_kwargs):
    """Compile + run on 8 cores. Returns (output[4], BassKernelResults)."""
    nc = _build_program()
    in_maps = _make_in_maps(**inputs)
    results = run_bass_kernel_spmd(
        nc, in_maps, core_ids=list(range(NCORES)), trace=trace, **run_kwargs)
    partial = np.float64(0.0)
    for r in results.results:
        partial += np.float64(r["out"][0, 0])
    h = np.float32(-(partial / N))
    entropy_loss = np.float32(BETA) * h
    target_entropy_loss = np.float32((h - TARGET_ENTROPY) ** 2)
    total_loss = entropy_loss + target_entropy_loss
    outv = np.stack([entropy_loss, target_entropy_loss, total_loss, h]).astype(
        np.float32)
    return outv, results


def kernel(**inputs):
    outv, _ = run(inputs, trace=False)
    return outv


# revision 50
# speedup vs baseline: 1.1690x; 1.0300x over previous
"""KNIFE entropy regularizer loss on 8 Trainium2 NeuronCores.

reference math (per token n, center k):
    dist_sq[n,k] = max(||x_n||^2 + ||c_k||^2 - 2 x_n.c_k, 0)
    kv[n,k]      = exp(-dist_sq / (2 s_k^2))
    density[n]   = sum_k w_k kv[n,k]
    h            = -mean_n log(density + EPS)
    out          = [BETA*h, (h-TGT)^2, BETA*h + (h-TGT)^2, h]

Sharding: data-parallel over the flattened token axis N = B*S = 8192,
1024 tokens per core, shard pre-transposed to [H=1024, T=1024] so the
contraction axis lands on SBUF partitions.  Tiny params are host-folded:
nu_k = -1/(2 s_k^2), bias_k = nu_k*||c_k||^2 + ln w_k, and -2c packed in
the [128, j, k] chunk layout the PE weights want.

Device pipeline per core (~30us vs the 33us cast-DMA baseline; wire
measured at 376GB/s aggregate over the 16 DMA engines):
  - 11 raw fp32 HWDGE DMAs on the SP queue (7 full 128-row H-chunks +
    the last chunk in 4 token-pieces [512/256/128/128] so the tail
    pipelines): no cast in flight, no SWDGE descriptor-gen serialization.
  - PE: psum[k,t] accumulates -2c.x via fp32r matmuls (full rate at
    >=256 moving columns, no input casts) and ||x||^2 via bf16
    ones-matmuls over DVE-squared chunks; PSUM is split into one tile
    per token-piece (whole-tile dep tracking would otherwise serialize
    the epilogue behind the last matmul); 5 filler matmuls after chunk 0
    hold the HAM clock ramp through the early DMA gap.
  - ACT/DVE: raw dist rows copied PSUM -> SBUF bf16 with a ones-row;
    the [K+1, K] transpose-matmul weights [diag(nu); nu*csq + ln w]
    apply the exp affine during the PE transpose to [128t, blk, k].
  - ACT: exp on [128, 80] (80 columns instead of 2x512), one table load
    (pre-placed InstLoadActFuncSet of the combined exp+ln table set).
  - DVE: density[t, blk] = free-axis reduce over k.
  - ACT: ln(density + EPS) on [128, 8] with per-partition accum_out.
  - PE: ones-matmul partition-sum -> [1,1]; the out DMA is ONE
    descriptor (a [128,1] out spends ~7us on per-descriptor completion
    semaphores at kernel end).
The max(dist,0) clamp is a no-op for this regime (dist ~ 1e3, exp
underflows to 0 either way) and is elided.
"""

from contextlib import ExitStack

import numpy as np

import concourse.bass as bass
import concourse.tile as tile
from concourse import bacc, mybir
from concourse.bass_utils import run_bass_kernel_spmd
from concourse.hw_specs import get_activation_tables

B, S, H, K = 4, 2048, 1024, 10
N = B * S                      # 8192 tokens
NCORES = 8
TPC = N // NCORES              # 1024 tokens per core
HCHUNKS = H // 128             # 8 chunks of 128 partitions
HALF = 512
NBLK = TPC // 128              # 8 token blocks for the transposed epilogue
BETA = 1.0
TARGET_ENTROPY = 0.0
EPS = 1e-8

F32 = mybir.dt.float32
F32R = mybir.dt.float32r
BF16 = mybir.dt.bfloat16



def _build_program():
    nc = bacc.Bacc("TRN2", target_bir_lowering=False, debug=False,
                   num_devices=NCORES)

    # x and -2c live as float32r end-to-end (same bits as fp32) so the PE
    # can consume them at full rate without a cast pass
    xT = nc.dram_tensor("xT", [H, TPC], F32R, kind="ExternalInput").ap()
    pc2 = nc.dram_tensor("pc2", [128, HCHUNKS * K], F32R,
                         kind="ExternalInput").ap()
    pf = nc.dram_tensor("pf", [K + 1, K], BF16, kind="ExternalInput").ap()
    out = nc.dram_tensor("out", [1, 1], F32, kind="ExternalOutput").ap()

    with tile.TileContext(nc) as tc, ExitStack() as ctx:
        _emit(tc, ctx, xT, pc2, pf, out)
    nc.compile()
    return nc


def _emit(tc, ctx, xT, pc2, pf, out):
    nc = tc.nc
    singles = ctx.enter_context(tc.tile_pool(name="singles", bufs=1))
    xpool = ctx.enter_context(tc.tile_pool(name="x", bufs=1))
    sqpool = ctx.enter_context(tc.tile_pool(name="sq", bufs=1))
    psum = ctx.enter_context(tc.tile_pool(name="ps", bufs=1, space="PSUM"))

    # ---- ACT: pin the combined exp+ln table once, before any activation;
    # Bacc.insert_act_table_loads honors the pre-placed load so neither Exp
    # nor Ln charges an implicit 1.3us table switch on the critical path ----
    table_id = list(get_activation_tables(nc.m.arch)).index(
        "natural_log_exp_and_others")
    nc.scalar.add_instruction(mybir.InstLoadActFuncSet(
        name=nc.get_next_instruction_name(), ins=[], outs=[],
        act_func_set_id=table_id))

    # ---- params on the ACT HWDGE queue: it carries no other DMAs, so
    # the completion semaphores process immediately (on the gpsimd SWDGE
    # queue they starve ~2us behind the x flood, delaying the first dot
    # matmul past chunk-0's arrival and resetting the PE clock ramp) ----
    c2_sb = singles.tile([128, HCHUNKS, K], F32R)
    nc.scalar.dma_start(c2_sb[:], pc2.rearrange("p (j k) -> p j k", k=K))
    ptw_sb = singles.tile([K + 1, K], BF16)
    nc.scalar.dma_start(ptw_sb[:], pf[:, :])
    c2 = c2_sb

    # ---- constants ----
    warm_rhs = singles.tile([128, HALF], BF16)
    nc.vector.memset(warm_rhs[:], 0.0)
    ones_bf = singles.tile([128, K], BF16)
    nc.vector.memset(ones_bf[:], 1.0)
    eps_sb = singles.tile([128, 1], F32)
    nc.vector.memset(eps_sb[:], EPS)
    zero_sb = singles.tile([128, 1], F32)
    nc.vector.memset(zero_sb[:], 0.0)

    # ---- x stream: 10 raw fp32 DMAs on the SP HWDGE queue.  Chunks 0-6
    # full [128, 1024]; chunk 7 in three token pieces so the tail of the
    # wire pipelines into the epilogue ----
    xb = [xpool.tile([128, TPC], F32R, name=f"xb_{j}", tag=f"x{j}")
          for j in range(HCHUNKS)]
    last = HCHUNKS - 1
    pieces = [(0, 512), (512, 768), (768, 896), (896, TPC)]
    for j in range(last):
        if j == 1:
            # chunk 1 rides the (otherwise idle) SWDGE queue: the SP
            # queue's completion pipeline backs up early and holds chunk
            # 1's semaphore ~2.3us past its bytes; a private queue fires
            # it at bytes+0.9
            nc.gpsimd.dma_start(xb[j][:], xT[j * 128:(j + 1) * 128, :])
        else:
            nc.sync.dma_start(xb[j][:], xT[j * 128:(j + 1) * 128, :])
    for (t0, t1) in pieces:
        nc.sync.dma_start(xb[last][:, t0:t1], xT[last * 128:, t0:t1])



    # ---- PE warmup: bridge idle -> data-gated stream at full clock ----
    ps_warm = psum.tile([K, HALF], F32)
    for _ in range(7):
        nc.tensor.matmul(ps_warm[:], lhsT=ones_bf[:], rhs=warm_rhs[:],
                         start=True, stop=True)

    # ---- main accumulation: psum[k, t] = x_sq[t] - 2 dot[k, t].
    # One PSUM tile per token region so each region's consumers release as
    # soon as its own writers finish (whole-tile dep tracking otherwise
    # serializes the epilogue behind the last matmul) ----
    psd = [psum.tile([16, t1 - t0], F32, name=f"psd_{i}")
           for i, (t0, t1) in enumerate(pieces)]

    def mm(out_ap, lhsT, rhs, **kw):
        nc.tensor.matmul(out_ap, lhsT=lhsT, rhs=rhs, skip_group_check=True,
                         **kw)

    sq = [sqpool.tile([128, TPC], BF16, name=f"sq_{j}", tag=f"s{j}")
          for j in range(HCHUNKS)]
    for j in range(last):
        xf = xb[j][:].bitcast(F32)
        nc.vector.tensor_mul(sq[j][:], xf, xf)
        c2j = c2[:, j, :]
        for i, (t0, t1) in enumerate(pieces):
            mm(psd[i][0:K, :], c2j, xb[j][:, t0:t1],
               start=(j == 0), stop=False)
        for i, (t0, t1) in enumerate(pieces):
            mm(psd[i][0:K, :], ones_bf[:], sq[j][:, t0:t1],
               start=False, stop=False)
        if j == 0:
            # the wire delivers chunk 1 ~2.4us after chunk 0; keep the PE
            # busy through the gap so the HAM clock ramp reaches 2.4GHz
            # instead of resetting (idle PE falls back to 1.2GHz and the
            # whole stream + tail then runs at half rate)
            for _ in range(5):
                nc.tensor.matmul(ps_warm[:], lhsT=ones_bf[:],
                                 rhs=warm_rhs[:], start=True, stop=True)

    # last chunk: per-piece so compute overlaps the DMA tail
    c2l = c2[:, last, :]
    for i, (t0, t1) in enumerate(pieces):
        sl = slice(t0, t1)
        xf = xb[last][:, sl].bitcast(F32)
        nc.vector.tensor_mul(sq[last][:, sl], xf, xf)
        mm(psd[i][0:K, :], c2l, xb[last][:, sl],
           start=False, stop=False)
        mm(psd[i][0:K, :], ones_bf[:], sq[last][:, sl],
           start=False, stop=True)

    # ---- epilogue, token-major.  The raw dist rows are copied PSUM->SBUF
    # in bf16 with a ones-row appended; the transpose matmul's weights
    # [K+1, K] = [diag(ninv); bias] then compute z = ninv*dist + bias
    # (bias = ninv*csq + ln w) as part of the transpose itself ----
    zd = [singles.tile([K + 1, t1 - t0], BF16, name=f"zd_{i}")
          for i, (t0, t1) in enumerate(pieces)]
    for i in range(len(pieces)):
        nc.vector.memset(zd[i][:], 1.0)      # row K stays the ones-row
    # pieces 0-1 copy on ACT (idle until the exps), 2-3 on DVE: neither
    # engine serializes more than ~0.6us of PSUM->SBUF copy on the tail
    nc.scalar.copy(zd[0][0:K, :], psd[0][0:K, :])
    nc.scalar.copy(zd[1][0:K, :], psd[1][0:K, :])
    nc.vector.tensor_copy(zd[2][0:K, :], psd[2][0:K, :])
    nc.vector.tensor_copy(zd[3][0:K, :], psd[3][0:K, :])

    # transpose z to [128t, blk, k] via tiny identity matmuls; exp and the
    # density reduce run per piece-group so piece 0-1 epilogue work is not
    # ordered behind piece 2's transposes (whole-tile dep tracking)
    psT = psum.tile([128, NBLK, K], F32)
    kvT = singles.tile([128, NBLK, K], F32)
    dens = singles.tile([128, NBLK], F32)
    blocks = [(t0 // 128, t1 // 128) for (t0, t1) in pieces]
    for i, (b0, b1) in enumerate(blocks):
        for b in range(b0, b1):
            o = (b - b0) * 128
            mm(psT[:, b, :], zd[i][:, o:o + 128], ptw_sb[:],
               start=True, stop=True)
        if b1 == 6:
            nc.scalar.activation(kvT[:, 0:6, :], psT[:, 0:6, :],
                                 mybir.ActivationFunctionType.Exp,
                                 bias=zero_sb[:])
            nc.vector.tensor_reduce(dens[:, 0:6], kvT[:, 0:6, :],
                                    axis=mybir.AxisListType.X,
                                    op=mybir.AluOpType.add)
    nc.scalar.activation(kvT[:, 6:NBLK, :], psT[:, 6:NBLK, :],
                         mybir.ActivationFunctionType.Exp, bias=zero_sb[:])
    nc.vector.tensor_reduce(dens[:, 6:NBLK], kvT[:, 6:NBLK, :],
                            axis=mybir.AxisListType.X, op=mybir.AluOpType.add)

    # ln(density + eps), accumulated along the free axis per partition
    lnout = singles.tile([128, NBLK], F32)
    lnacc = singles.tile([128, 1], F32)
    nc.scalar.activation(lnout[:], dens[:], mybir.ActivationFunctionType.Ln,
                         bias=eps_sb[:], accum_out=lnacc[:])

    # partition-sum on the PE so the output DMA is ONE descriptor (a
    # [128, 1] DMA is 128 4-byte descriptors whose completion-semaphore
    # processing alone costs ~7us at kernel end); fp32 keeps the partial
    # sums exact (bf16 here costs a 5e-3 systematic rounding bias)
    ones_f32 = singles.tile([128, 1], F32)
    nc.vector.memset(ones_f32[:], 1.0)
    ps_sum = psum.tile([1, 1], F32)
    nc.tensor.matmul(ps_sum[:], lhsT=lnacc[:], rhs=ones_f32[:],
                     start=True, stop=True)
    total_sb = singles.tile([1, 1], F32)
    nc.vector.tensor_copy(total_sb[:], ps_sum[:])
    nc.scalar.dma_start(out[:, :], total_sb[:])


def _make_in_maps(hidden_states, kernel_centers, kernel_weights, kernel_scales):
    h_flat = np.asarray(hidden_states, dtype=np.float32).reshape(N, H)
    c = np.asarray(kernel_centers, np.float64)
    w = np.asarray(kernel_weights, np.float64).reshape(K)
    s = np.asarray(kernel_scales, np.float64).reshape(K)

    ninv = -1.0 / (2.0 * s * s)
    csq = np.sum(c * c, axis=1)                       # [K]
    with np.errstate(divide="ignore"):
        lnw = np.log(w)                               # w=0 -> -inf -> e^z=0
    zbias = ninv * csq + lnw

    # [p, j*K+k] = -2 c[k, j*128+p]
    pc2 = np.ascontiguousarray(
        (-2.0 * c.T).reshape(HCHUNKS, 128, K).transpose(1, 0, 2).reshape(
            128, HCHUNKS * K).astype(np.float32))
    # transpose-matmul weights: out[t, k] = dist[k, t]*ninv_k + bias_k
    import ml_dtypes
    pf = np.zeros((K + 1, K), np.float32)
    pf[:K, :] = np.diag(ninv)
    pf[K, :] = zbias
    pf = pf.astype(ml_dtypes.bfloat16)

    in_maps = []
    for core in range(NCORES):
        shard = h_flat[core * TPC:(core + 1) * TPC, :]    # [TPC, H]
        in_maps.append({
            "xT": np.ascontiguousarray(shard.T),          # [H, TPC]
            "pc2": pc2,
            "pf": pf,
        })
    return in_maps


def run(inputs, trace=False, **run_kwargs):
    """Compile + run on 8 cores. Returns (output[4], BassKernelResults)."""
    nc = _build_program()
    in_maps = _make_in_maps(**inputs)
    results = run_bass_kernel_spmd(
        nc, in_maps, core_ids=list(range(NCORES)), trace=trace, **run_kwargs)
    partial = np.float64(0.0)
    for r in results.results:
        partial += np.float64(r["out"][0, 0])
    h = np.float32(-(partial / N))
    entropy_loss = np.float32(BETA) * h
    target_entropy_loss = np.float32((h - TARGET_ENTROPY) ** 2)
    total_loss = entropy_loss + target_entropy_loss
    outv = np.stack([entropy_loss, target_entropy_loss, total_loss, h]).astype(
        np.float32)
    return outv, results


def kernel(**inputs):
    outv, _ = run(inputs, trace=False)
    return outv
